# revision 22
# baseline (speedup 1.0000x reference)
"""Harris corner NMS kernel for 8 TRN2 NeuronCores (Bass/Tile).

Single device phase per core (512-row shard, 7-row/4-col halo):
  Sobel: horizontal diff/smooth on DVE/GpSimd, vertical taps as one banded
  fp32 matmul each (K-contraction of a [128,128] band). Products on ACT/DVE.
  7x7 gauss exploits the rank-1 kernel: symmetric horizontal pair-sums
  (DVE/GpSimd) + 2 PSUM-accumulated band matmuls per channel (vs 7 direct).
  R-combine on ACT+DVE. 7x7 maxpool in-tile: 3 log-shifted horizontal maxes,
  vertical maxes in PE-transposed space; cand = R * (R == maxpool(R)).
  Outputs R (for the host median) and cand.
Host: exact lower-median via np.partition; patches 3 R border rows and 7
  cand border rows (reference zero-pads products at image edges); final
  out = where(cand >= med, cand, 0) - valid since med > 0 makes the
  median-threshold commute with the NMS equality mask.
"""
import os
import sys
import tempfile

import numpy as np

sys.path.insert(0, "/opt/trn_rl_repo")

from contextlib import ExitStack

import concourse.bass as bass
import concourse.tile as tile
from concourse import bacc, mybir
from concourse.bass_utils import run_bass_kernel_spmd

F32 = mybir.dt.float32
H = W = 4096
NC = 8
RPC = H // NC          # 512 rows per core
ALPHA = np.float32(0.05)
NEG = np.float32(-3.0e38)
P1_STRIDE = 114        # valid out rows per tile (sobel 1 + gauss 3 + pool 3)
P2_STRIDE = 122        # valid out rows per 128-row tile (maxpool 3 halo)
WP1 = W + 8            # x cols padded by 4 each side
WPP = W + 8            # product cols (same coords as padded x)
WP2 = W + 6            # R cols padded by 3 each side
XROWS = RPC + 14       # 526 x-rows per core (halo 7 each side)
RROWS = RPC + 6        # 518 R-rows per core (phase 2 input)

_CACHE = {}


def _band(taps, off):
    B = np.zeros((128, 128), np.float32)
    idx = np.arange(128)
    for j, t in enumerate(taps):
        d = j - off  # p - m = d
        if t != 0.0:
            m = idx[max(0, -d): 128 - max(0, d)]
            B[m + d, m] = t
    return B


def _host_bands(gk):
    """4 bands: vertical smooth/diff for sobel + 2 scaled gauss bands.

    gk is rank-1 (outer(g1,g1)/Z) to ~1e-7: factor the 7x7 conv as
    S = Bg @ (P0 + r2*pr1) + (r1*Bg) @ (pr2 + (r0/r1)*pr3), where
    pr_j = P[.-j] + P[.+j] (horizontal symmetric pair sums on DVE) and
    Bg = band(gk[:,3]) carries the vertical taps.
    """
    sy = _band(np.array([1.0, 2.0, 1.0], np.float32), 1)
    dy = _band(np.array([-1.0, 0.0, 1.0], np.float32), 1)
    gv = gk[:, 3].astype(np.float32)
    r = (gk[3, :] / gk[3, 3]).astype(np.float32)
    bg1 = _band(gv, 3)
    bg2 = _band((r[1] * gv).astype(np.float32), 3)
    scal = (float(r[2]), float(r[0] / r[1]))
    return np.ascontiguousarray(np.stack([sy, dy, bg1, bg2])), scal  # [4,128,128]


def _build_phase1(scal):
    r2, r01 = scal
    nc = bacc.Bacc("TRN2", target_bir_lowering=False, debug=False, num_devices=NC)
    xs = nc.dram_tensor("xs", [XROWS, WP1], F32, kind="ExternalInput").ap()
    bands = nc.dram_tensor("bands", [4, 128, 128], F32, kind="ExternalInput").ap()
    idin = nc.dram_tensor("ident", [128, 128], F32, kind="ExternalInput").ap()
    r_out = nc.dram_tensor("r", [RPC, W], F32, kind="ExternalOutput").ap()
    cand_out = nc.dram_tensor("cand", [RPC, W], F32, kind="ExternalOutput").ap()

    NT = 5                      # row tiles: 4 full (120 valid) + 1 partial (32)
    CH = 512                    # col chunk
    with tile.TileContext(nc) as tc, ExitStack() as ctx:
        wpool = ctx.enter_context(tc.tile_pool(name="w", bufs=1))
        xpool = ctx.enter_context(tc.tile_pool(name="x", bufs=1))
        ppool = ctx.enter_context(tc.tile_pool(name="p", bufs=1))
        rpool = ctx.enter_context(tc.tile_pool(name="r", bufs=2))
        tpool = ctx.enter_context(tc.tile_pool(name="t", bufs=1))
        gpool = ctx.enter_context(tc.tile_pool(name="g", bufs=2))
        psum_i = ctx.enter_context(
            tc.tile_pool(name="psi", bufs=1, space=bass.MemorySpace.PSUM))
        psum_s = ctx.enter_context(
            tc.tile_pool(name="pss", bufs=1, space=bass.MemorySpace.PSUM))
        psum_t = ctx.enter_context(
            tc.tile_pool(name="pst", bufs=2, space=bass.MemorySpace.PSUM))
        mxpool = ctx.enter_context(tc.tile_pool(name="mx", bufs=1))
        cpool = ctx.enter_context(tc.tile_pool(name="cnd", bufs=1))

        wsb = wpool.tile([128, 4, 128], F32)
        nc.sync.dma_start(wsb[:], bands.rearrange("k p m -> p k m"))
        ident = wpool.tile([128, 128], F32)
        nc.sync.dma_start(ident[:], idin)

        def bandw(j):
            return wsb[:, j, :]

        for t in range(NT):
            r0 = t * P1_STRIDE                    # first valid out row (local)
            K = min(128, XROWS - r0)              # input rows this tile
            nv = min(P1_STRIDE, RPC - r0)         # valid cand rows
            hi = min(124, K - 4)                  # R rows valid in 4..hi-1
            xt = xpool.tile([128, WP1], F32, tag="xt")
            nc.sync.dma_start(xt[:K, :], xs[r0:r0 + K, :])

            Ps = [ppool.tile([128, WPP], F32, tag=f"P{ch}", name=f"P{ch}") for ch in range(3)]
            # products loop over P cols q in [1, WPP-1)
            qs_list = [(1 + i * CH, CH) for i in range(8)] + [(1 + 8 * CH, WPP - 2 - 8 * CH)]
            for (q0, wch) in qs_list:
                # horizontal sobel parts on DVE: d = x[+1]-x[-1], sm = 2x+x[-1]+x[+1]
                dch = gpool.tile([128, CH], F32, tag="dch", name="dch")
                tch = gpool.tile([128, CH], F32, tag="tch", name="tch")
                sch = gpool.tile([128, CH], F32, tag="sch", name="sch")
                nc.vector.tensor_tensor(
                    dch[:K, :wch], xt[:K, q0 + 1:q0 + 1 + wch],
                    xt[:K, q0 - 1:q0 - 1 + wch], mybir.AluOpType.subtract)
                nc.gpsimd.tensor_tensor(
                    tch[:K, :wch], xt[:K, q0 - 1:q0 - 1 + wch],
                    xt[:K, q0 + 1:q0 + 1 + wch], mybir.AluOpType.add)
                nc.vector.scalar_tensor_tensor(
                    sch[:K, :wch], xt[:K, q0:q0 + wch], 2.0, tch[:K, :wch],
                    mybir.AluOpType.mult, mybir.AluOpType.add)
                ixp = psum_i.tile([128, CH], F32, tag="ix")
                iyp = psum_i.tile([128, CH], F32, tag="iy")
                nc.tensor.matmul(ixp[:, :wch], bandw(0)[:K], dch[:K, :wch],
                                 start=True, stop=True)
                nc.tensor.matmul(iyp[:, :wch], bandw(1)[:K], sch[:K, :wch],
                                 start=True, stop=True)
                nc.scalar.activation(Ps[0][:, q0:q0 + wch], ixp[:, :wch],
                                     mybir.ActivationFunctionType.Square)
                nc.scalar.activation(Ps[1][:, q0:q0 + wch], iyp[:, :wch],
                                     mybir.ActivationFunctionType.Square)
                iysb = tpool.tile([128, CH], F32, tag="iysb", name="iysb")
                nc.scalar.activation(iysb[:, :wch], iyp[:, :wch],
                                     mybir.ActivationFunctionType.Copy)
                nc.vector.tensor_tensor(Ps[2][:, q0:q0 + wch], ixp[:, :wch],
                                        iysb[:, :wch], mybir.AluOpType.mult)
            # zero products outside the image (cols): img col c <-> P col c+4
            for ch in range(3):
                nc.gpsimd.memset(Ps[ch][:, 1:4], 0.0)
                nc.gpsimd.memset(Ps[ch][:, W + 4:W + 7], 0.0)

            rsb = rpool.tile([128, W + 6], F32, tag="rsb")
            nc.gpsimd.memset(rsb[:, 0:3], float(NEG))
            nc.gpsimd.memset(rsb[:, W + 3:W + 6], float(NEG))
            for c in range(8):
                c0 = c * CH
                sps = [psum_s.tile([128, CH], F32, tag=f"s{ch}", name=f"s{ch}") for ch in range(3)]
                # gauss: S = Bg1 @ (P0 + r2*pr1) + Bg2 @ (pr2 + r01*pr3)
                for ch in range(3):
                    P = Ps[ch]
                    pr1 = gpool.tile([128, CH], F32, tag="pr1", name="pr1")
                    pr2 = gpool.tile([128, CH], F32, tag="pr2", name="pr2")
                    pr3 = gpool.tile([128, CH], F32, tag="pr3", name="pr3")
                    t1 = gpool.tile([128, CH], F32, tag="gt1", name="gt1")
                    t2 = gpool.tile([128, CH], F32, tag="gt2", name="gt2")
                    nc.gpsimd.tensor_tensor(pr1[:K], P[:K, c0 + 3:c0 + 3 + CH],
                                            P[:K, c0 + 5:c0 + 5 + CH], mybir.AluOpType.add)
                    nc.gpsimd.tensor_tensor(pr2[:K], P[:K, c0 + 2:c0 + 2 + CH],
                                            P[:K, c0 + 6:c0 + 6 + CH], mybir.AluOpType.add)
                    nc.gpsimd.tensor_tensor(pr3[:K], P[:K, c0 + 1:c0 + 1 + CH],
                                            P[:K, c0 + 7:c0 + 7 + CH], mybir.AluOpType.add)
                    nc.vector.scalar_tensor_tensor(
                        t1[:K], pr1[:K], float(r2), P[:K, c0 + 4:c0 + 4 + CH],
                        mybir.AluOpType.mult, mybir.AluOpType.add)
                    nc.vector.scalar_tensor_tensor(
                        t2[:K], pr3[:K], float(r01), pr2[:K],
                        mybir.AluOpType.mult, mybir.AluOpType.add)
                    nc.tensor.matmul(sps[ch][:], bandw(2)[:K], t1[:K],
                                     start=True, stop=False)
                    nc.tensor.matmul(sps[ch][:], bandw(3)[:K], t2[:K],
                                     start=False, stop=True)
                a, b, cc = sps
                t1 = tpool.tile([128, CH], F32, tag="t1")
                t1s = tpool.tile([128, CH], F32, tag="t1s")
                u = tpool.tile([128, CH], F32, tag="u")
                v = tpool.tile([128, CH], F32, tag="v")
                w_ = tpool.tile([128, CH], F32, tag="w")
                asb = tpool.tile([128, CH], F32, tag="asb", name="asb")
                nc.scalar.activation(asb[:hi], a[:hi],
                                     mybir.ActivationFunctionType.Copy)
                nc.vector.tensor_tensor(t1[:hi], asb[:hi], b[:hi],
                                        mybir.AluOpType.add)
                nc.scalar.activation(t1s[:hi], t1[:hi],
                                     mybir.ActivationFunctionType.Square)
                nc.vector.tensor_tensor(u[:hi], asb[:hi], b[:hi],
                                        mybir.AluOpType.mult)
                nc.scalar.activation(v[:hi], cc[:hi],
                                     mybir.ActivationFunctionType.Square)
                nc.vector.scalar_tensor_tensor(
                    w_[:hi], t1s[:hi], float(-ALPHA), u[:hi],
                    mybir.AluOpType.mult, mybir.AluOpType.add)
                nc.vector.tensor_tensor(rsb[:hi, 3 + c0:3 + c0 + CH], w_[:hi],
                                        v[:hi], mybir.AluOpType.subtract)
            nc.sync.dma_start(r_out[r0:r0 + nv, :], rsb[7:7 + nv, 3:W + 3])
            # 7x7 maxpool of R and cand = R * (R == Mp)
            m1 = mxpool.tile([128, W + 5], F32, tag="hA")
            nc.vector.tensor_tensor(m1[:hi], rsb[:hi, 0:W + 5], rsb[:hi, 1:W + 6],
                                    mybir.AluOpType.max)
            m2 = mxpool.tile([128, W + 3], F32, tag="hB")
            nc.vector.tensor_tensor(m2[:hi], m1[:hi, 0:W + 3], m1[:hi, 2:W + 5],
                                    mybir.AluOpType.max)
            m3 = mxpool.tile([128, W], F32, tag="hA")
            nc.vector.tensor_tensor(m3[:hi], m2[:hi, 0:W], m2[:hi, 3:W + 3],
                                    mybir.AluOpType.max)
            candt = cpool.tile([128, W], F32, tag="candt")
            for g in range(8):
                mT = mxpool.tile([128, 4, 128], F32, tag=f"mT{g % 2}", name="mT")
                for j in range(4):
                    cch = 4 * g + j
                    tp = psum_t.tile([128, 128], F32, tag="tv", name="tp")
                    nc.tensor.transpose(tp[:, :hi], m3[:hi, cch * 128:(cch + 1) * 128],
                                        ident[:hi, :hi])
                    nc.scalar.activation(mT[:, j, :hi], tp[:, :hi],
                                         mybir.ActivationFunctionType.Copy)
                v1 = mxpool.tile([128, 4, 128], F32, tag="v1", name="v1")
                nc.vector.tensor_tensor(v1[:, :, 0:hi - 1], mT[:, :, 0:hi - 1],
                                        mT[:, :, 1:hi], mybir.AluOpType.max)
                v2 = mxpool.tile([128, 4, 128], F32, tag="v2", name="v2")
                nc.vector.tensor_tensor(v2[:, :, 0:hi - 3], v1[:, :, 0:hi - 3],
                                        v1[:, :, 2:hi - 1], mybir.AluOpType.max)
                v3 = mxpool.tile([128, 4, 128], F32, tag="v3", name="v3")
                nc.vector.tensor_tensor(v3[:, :, 3:hi - 3], v2[:, :, 0:hi - 6],
                                        v2[:, :, 3:hi - 3], mybir.AluOpType.max)
                for j in range(4):
                    cch = 4 * g + j
                    vn = psum_t.tile([128, 128], F32, tag="tv", name="vn")
                    nc.tensor.transpose(vn[:hi, :], v3[:, j, :hi], ident[:, :])
                    sl = slice(3 + cch * 128, 3 + (cch + 1) * 128)
                    msk = mxpool.tile([128, 128], F32, tag="msk", name="msk")
                    nc.vector.tensor_tensor(msk[:hi], rsb[:hi, sl], vn[:hi, :],
                                            mybir.AluOpType.is_equal)
                    nc.gpsimd.tensor_tensor(candt[:hi, cch * 128:(cch + 1) * 128],
                                            msk[:hi], rsb[:hi, sl],
                                            mybir.AluOpType.mult)
            nc.sync.dma_start(cand_out[r0:r0 + nv, :], candt[7:7 + nv, :])
    nc.compile()
    return nc


def _build_phase2():
    nc = bacc.Bacc("TRN2", target_bir_lowering=False, debug=False, num_devices=NC)
    rs = nc.dram_tensor("rs", [RROWS, WP2], F32, kind="ExternalInput").ap()
    med = nc.dram_tensor("med", [128, 1], F32, kind="ExternalInput").ap()
    idin = nc.dram_tensor("ident", [128, 128], F32, kind="ExternalInput").ap()
    o_out = nc.dram_tensor("o", [RPC, W], F32, kind="ExternalOutput").ap()

    NT = 5
    with tile.TileContext(nc) as tc, ExitStack() as ctx:
        mpool = ctx.enter_context(tc.tile_pool(name="m", bufs=1))
        pool = ctx.enter_context(tc.tile_pool(name="p", bufs=1))
        pool2 = ctx.enter_context(tc.tile_pool(name="p2", bufs=2))
        psum2 = ctx.enter_context(
            tc.tile_pool(name="ps2", bufs=4, space=bass.MemorySpace.PSUM))
        msb = mpool.tile([128, 1], F32)
        nc.sync.dma_start(msb[:], med[:])
        ident = mpool.tile([128, 128], F32)
        nc.sync.dma_start(ident[:], idin[:])
        for t in range(NT):
            r0 = t * P2_STRIDE
            K = min(128, RROWS - r0)
            nv = min(P2_STRIDE, RPC - t * P2_STRIDE)
            rt = pool2.tile([128, WP2], F32, tag="rt")
            nc.sync.dma_start(rt[:K, :], rs[r0:r0 + K, :])
            th = pool2.tile([128, WP2], F32, tag="th")
            # threshold: th = (rt >= med) * rt   (pad rows/cols handled below / host)
            nc.vector.scalar_tensor_tensor(
                th[:K], rt[:K], msb[:K], rt[:K],
                mybir.AluOpType.is_ge, mybir.AluOpType.mult)
            nc.gpsimd.memset(th[:K, 0:3], float(NEG))
            nc.gpsimd.memset(th[:K, W + 3:W + 6], float(NEG))
            # horizontal running max, span 7 (down-anchored)
            m1 = pool.tile([128, WP2 - 1], F32, tag="A")
            nc.vector.tensor_tensor(m1[:K], th[:K, 0:WP2 - 1], th[:K, 1:WP2],
                                    mybir.AluOpType.max)
            m2 = pool.tile([128, WP2 - 3], F32, tag="B")
            nc.vector.tensor_tensor(m2[:K], m1[:K, 0:WP2 - 3], m1[:K, 2:WP2 - 1],
                                    mybir.AluOpType.max)
            m3 = pool.tile([128, W], F32, tag="A")
            nc.vector.tensor_tensor(m3[:K], m2[:K, 0:W], m2[:K, 3:W + 3],
                                    mybir.AluOpType.max)
            # vertical running max, span 7, per 4-block group so the whole
            # chain (transpose -> vmax -> transpose back -> mask) pipelines
            # across groups (engines cannot read SBUF/PSUM at a nonzero
            # partition base; free-dim shifts are legal in transposed space).
            NG = W // 512  # 8 groups of 4x128 cols
            ot = pool.tile([128, W], F32, tag="ot")
            for g in range(NG):
                mT = pool.tile([128, 4, 128], F32, tag=f"mT{g % 2}", name="mT")
                for j in range(4):
                    cch = 4 * g + j
                    tp = psum2.tile([128, 128], F32, tag="tp", name="tp")
                    nc.tensor.transpose(tp[:, :K], m3[:K, cch * 128:(cch + 1) * 128],
                                        ident[:K, :K])
                    nc.scalar.activation(mT[:, j, :K], tp[:, :K],
                                         mybir.ActivationFunctionType.Copy)
                v1 = pool.tile([128, 4, 128], F32, tag="v1", name="v1")
                nc.vector.tensor_tensor(v1[:, :, 0:K - 1], mT[:, :, 0:K - 1],
                                        mT[:, :, 1:K], mybir.AluOpType.max)
                v2 = pool.tile([128, 4, 128], F32, tag="v2", name="v2")
                nc.vector.tensor_tensor(v2[:, :, 0:K - 3], v1[:, :, 0:K - 3],
                                        v1[:, :, 2:K - 1], mybir.AluOpType.max)
                v3 = pool.tile([128, 4, 128], F32, tag="v3", name="v3")
                # place result at free index i+3 so transpose-back aligns with th rows
                nc.vector.tensor_tensor(v3[:, :, 3:K - 3], v2[:, :, 0:K - 6],
                                        v2[:, :, 3:K - 3], mybir.AluOpType.max)
                for j in range(4):
                    cch = 4 * g + j
                    vn = psum2.tile([128, 128], F32, tag="vn", name="vn")
                    nc.tensor.transpose(vn[:K, :], v3[:, j, :K], ident[:, :])
                    sl = slice(3 + cch * 128, 3 + (cch + 1) * 128)
                    msk = pool.tile([128, 128], F32, tag="msk", name="msk")
                    nc.vector.tensor_tensor(msk[:K], th[:K, sl], vn[:K, :],
                                            mybir.AluOpType.is_equal)
                    nc.gpsimd.tensor_tensor(ot[:K, cch * 128:(cch + 1) * 128],
                                            msk[:K], th[:K, sl],
                                            mybir.AluOpType.mult)
            nc.sync.dma_start(o_out[r0:r0 + nv, :], ot[3:3 + nv, :])
    nc.compile()
    return nc


def _conv2_same(img, ker, pad):
    kh, kw = ker.shape
    ip = np.pad(img, pad).astype(np.float32)
    out = np.zeros(img.shape, np.float32)
    for i in range(kh):
        for j in range(kw):
            out += ker[i, j] * ip[i:i + img.shape[0], j:j + img.shape[1]]
    return out


def _host_R_strip(x2d, gk, top):
    """Exact reference R for the top (top=True) or bottom 3 rows, full width.

    Uses a 16-row slab touching the true image edge so the zero-padding of
    both the sobel input and the products matches the reference; only rows
    >=4 away from the slab's interior cut are kept (3 needed, 11 valid).
    """
    slab = x2d[:16] if top else x2d[-16:]
    sob = np.array([[-1., 0., 1.], [-2., 0., 2.], [-1., 0., 1.]], np.float32)
    Ix = _conv2_same(slab, sob, 1)
    Iy = _conv2_same(slab, sob.T, 1)
    a = _conv2_same(Ix * Ix, gk, 3)
    b = _conv2_same(Iy * Iy, gk, 3)
    c = _conv2_same(Ix * Iy, gk, 3)
    tr = a + b
    Rs = a * b - c * c - ALPHA * tr * tr
    return Rs[:3] if top else Rs[-3:]


def _host_border_fix(out, Rt, rows):
    """Recompute maxpool+mask for the given rows exactly on host."""
    Rp = np.pad(Rt, 3, constant_values=-np.inf)
    for r in rows:
        m = np.full(W, -np.inf, np.float32)
        for i in range(7):
            for j in range(7):
                m = np.maximum(m, Rp[r + i, j:j + W])
        row = Rt[r]
        out[r] = row * (row == m)
    return out


def _ensure_ntff_hook():
    """The agent image's antenv lacks axon_hooks; inject it so trace=True works."""
    try:
        import antenv.axon_hooks  # noqa: F401
        return
    except ImportError:
        pass
    try:
        import types
        import antenv
        from trn_agent_boot.trn_boot import _ntff_profile_via_ctypes
        mod = types.ModuleType("antenv.axon_hooks")
        _state = {"hook": None}
        mod.set_axon_ntff_profile_hook = lambda h: _state.__setitem__("hook", h)
        mod.get_axon_ntff_profile_hook = lambda: _state["hook"]
        sys.modules["antenv.axon_hooks"] = mod
        antenv.axon_hooks = mod
        mod.set_axon_ntff_profile_hook(
            _ntff_profile_via_ctypes("/opt/axon/libaxon_pjrt.so"))
    except Exception as e:  # profiling is best-effort
        print(f"ntff hook setup failed: {e}")


def kernel(x, gauss_kernel):
    x2d = np.ascontiguousarray(np.asarray(x, np.float32).reshape(H, W))
    gk = np.asarray(gauss_kernel, np.float32).reshape(7, 7)

    bands, scal = _host_bands(gk)
    if _CACHE.get("scal") != scal:
        _CACHE.pop("p1", None)
        _CACHE["scal"] = scal
    if "p1" not in _CACHE:
        _CACHE["p1"] = _build_phase1(scal)
    nc1 = _CACHE["p1"]
    xp = np.pad(x2d, ((7, 7), (4, 4)))
    ident = np.eye(128, dtype=np.float32)
    in_maps1 = [
        {"xs": np.ascontiguousarray(xp[c * RPC: c * RPC + XROWS]), "bands": bands,
         "ident": ident}
        for c in range(NC)
    ]
    trace = bool(int(os.environ.get("KERNEL_TRACE", "0")))
    if trace:
        _ensure_ntff_hook()
    res1 = run_bass_kernel_spmd(nc1, in_maps1, core_ids=list(range(NC)), trace=trace)
    _CACHE["t1"] = res1.exec_time_ns
    _CACHE["t2"] = 0
    R = np.concatenate([res1.results[c]["r"] for c in range(NC)], axis=0)
    cand = np.concatenate([res1.results[c]["cand"] for c in range(NC)], axis=0)

    # patch 3-row borders with exact reference semantics (zero-padded products)
    R[:3] = _host_R_strip(x2d, gk, True)
    R[-3:] = _host_R_strip(x2d, gk, False)

    k = (R.size - 1) // 2
    med = np.partition(R.ravel(), k)[k]

    # patch cand rows affected by the R border patch (7 top/bottom)
    Rp = np.pad(R, 3, constant_values=-np.inf)
    for rows in (range(0, 7), range(H - 7, H)):
        for r in rows:
            m = np.full(W, -np.inf, np.float32)
            for i in range(7):
                for j in range(7):
                    m = np.maximum(m, Rp[r + i, j:j + W])
            cand[r] = R[r] * (R[r] == m)

    if med > 0:
        out = np.where(cand >= med, cand, np.float32(0)).astype(np.float32)
    else:  # fallback: exact reference semantics on host (never hit for this input)
        Rt = np.where(R < med, np.float32(0), R).astype(np.float32)
        Rtp = np.pad(Rt, 3, constant_values=-np.inf)
        Rmax = np.full_like(Rt, -np.inf)
        for i in range(7):
            for j in range(7):
                Rmax = np.maximum(Rmax, Rtp[i:i + H, j:j + W])
        out = (Rt * (Rt == Rmax)).astype(np.float32)
    return out.reshape(1, 1, H, W)

